# revision 2
# baseline (speedup 1.0000x reference)
"""Trainium2 Bass kernel for the BalSCL/SSL balanced supervised-contrastive loss.

Distribution: data-parallel over the 8192 anchor rows, 1024 rows per core on
8 NeuronCores.  Each core computes a partial loss numerator / denominator and
the host combines the 8 scalar pairs.

Math (restructured from the reference, analytically identical):
  N = 8292 columns (8192 anchors + 100 class centers), all unit-norm.
  The row-max subtraction in the reference cancels analytically, so
    loss_i = log(S_i) - (10/m_i) * Sm_i
  with
    S_i  = sum_{j != i} exp(10 * f_i . g_j) / (cc_j - [lab_j == lab_i])
    Sm_i = sum_{j != i, lab_j == lab_i} f_i . g_j
    m_i  = cc[lab_i] - 1      (number of positive pairs for row i)
  Using one-hot structure everything reduces to per-class aggregates computed
  on the tensor engine:
    E[c, i]  = sum_{j in class c} exp(10 * rawT[j, i])     (incl. j == i)
    gsum[:, c] = sum_{j in class c} g_j ;  A[c, i] = gsum[:, c] . f_i
  and the diagonal (j == i) contribution is subtracted analytically using
  ||f_i||^2, re-quantized to bf16 so it matches the bf16-stored exp that
  entered E bit-for-bit.
"""

import sys

sys.path.insert(0, "/opt/trn_rl_repo")

import numpy as np
import ml_dtypes

import concourse.bass as bass  # noqa: F401  (import keeps bass registered)
import concourse.bacc as bacc
import concourse.tile as tile
from concourse import mybir
from concourse.bass_utils import run_bass_kernel_spmd

F32 = mybir.dt.float32
BF16 = mybir.dt.bfloat16
BF = ml_dtypes.bfloat16
AF = mybir.ActivationFunctionType

B2, C, D = 8192, 100, 128
TEMP = 0.1
N = B2 + C                # 8292
TJ = (N + 127) // 128     # 65 j-tiles
NPAD = TJ * 128           # 8320
CORES = 8
R = B2 // CORES           # 1024 rows per core
CH = 512                  # i-chunk width (fp32 PSUM bank)
NCH = R // CH             # 2 chunks per core
GW = 3                    # j-tiles per exp group (3 PSUM banks)
GROUPS = [(g * GW, min(GW, TJ - g * GW)) for g in range((TJ + GW - 1) // GW)]

_NC_CACHE = {}


def _build_nc():
    nc = bacc.Bacc()

    fTg = nc.dram_tensor("fTg", [D, NPAD], BF16, kind="ExternalInput")
    fAn = nc.dram_tensor("fAn", [128, TJ * 128], BF16, kind="ExternalInput")
    TAg = nc.dram_tensor("TAg", [128, TJ * C], BF16, kind="ExternalInput")
    fTc = nc.dram_tensor("fTc", [D, R], BF16, kind="ExternalInput")
    tTp = nc.dram_tensor("tTp", [C, R], F32, kind="ExternalInput")
    W2 = nc.dram_tensor("W2", [C, R], F32, kind="ExternalInput")
    conf = nc.dram_tensor("conf", [1, R], F32, kind="ExternalInput")
    ccm1 = nc.dram_tensor("ccm1", [C, 1], F32, kind="ExternalInput")
    outd = nc.dram_tensor("out", [1, 2], F32, kind="ExternalOutput")

    with tile.TileContext(nc) as tc:
        with (
            tc.tile_pool(name="consts", bufs=1) as cp,
            tc.tile_pool(name="expp", bufs=3) as ep,
            tc.tile_pool(name="asmp", bufs=2) as am,
            tc.tile_pool(name="rawp", bufs=2, space="PSUM") as rp,
            tc.tile_pool(name="epsp", bufs=1, space="PSUM") as pp,
            tc.tile_pool(name="smp", bufs=1, space="PSUM") as sp,
        ):
            # ---------------- input loads ----------------
            s_fTc = cp.tile([D, R], BF16)
            nc.sync.dma_start(out=s_fTc, in_=fTc[:])
            s_fTg = cp.tile([D, NPAD], BF16)
            for i in range(5):
                c0 = i * 13 * 128
                c1 = min(NPAD, c0 + 13 * 128)
                nc.sync.dma_start(out=s_fTg[:, c0:c1], in_=fTg[:, c0:c1])
            s_TAg = cp.tile([128, TJ * C], BF16)
            for i in range(3):
                c0 = i * 22 * C
                c1 = min(TJ * C, c0 + 22 * C)
                nc.sync.dma_start(out=s_TAg[:, c0:c1], in_=TAg[:, c0:c1])
            s_fAn = cp.tile([128, TJ * 128], BF16)
            for i in range(4):
                c0 = i * 17 * 128
                c1 = min(TJ * 128, c0 + 17 * 128)
                nc.sync.dma_start(out=s_fAn[:, c0:c1], in_=fAn[:, c0:c1])
            s_tTp = cp.tile([C, R], F32)
            nc.sync.dma_start(out=s_tTp, in_=tTp[:])
            s_W2 = cp.tile([C, R], F32)
            nc.sync.dma_start(out=s_W2, in_=W2[:])
            s_conf = cp.tile([1, R], F32)
            nc.sync.dma_start(out=s_conf, in_=conf[:])
            s_ccm1 = cp.tile([C, 1], F32)
            nc.sync.dma_start(out=s_ccm1, in_=ccm1[:])
            s_ones = cp.tile([128, 1], F32)
            nc.vector.memset(s_ones, 1.0)

            s_gsum = cp.tile([D, C], BF16)
            s_Scorr = cp.tile([1, R], F32)
            s_SmT = cp.tile([1, R], F32)

            # ------------- per-chunk raw/exp/E pipeline -------------
            def chunk_body(k):
                i0 = k * CH
                EPS = pp.tile([C, CH], F32, name=f"EPS{k}", tag="EPS")
                for (t0, gw) in GROUPS:
                    rawPS = rp.tile([128, CH * GW], F32, name="rawPS", tag="raw")
                    for q in range(gw):
                        t = t0 + q
                        nc.tensor.matmul(
                            rawPS[:, CH * q : CH * (q + 1)],
                            lhsT=s_fTg[:, 128 * t : 128 * (t + 1)],
                            rhs=s_fTc[:, i0 : i0 + CH],
                            start=True,
                            stop=True,
                        )
                    exps = ep.tile([128, CH * GW], BF16, name="exps", tag="exps")
                    nc.scalar.activation(
                        out=exps[:, : CH * gw],
                        in_=rawPS[:, : CH * gw],
                        func=AF.Exp,
                        scale=1.0 / TEMP,
                    )
                    for q in range(gw):
                        t = t0 + q
                        nc.tensor.matmul(
                            EPS,
                            lhsT=s_TAg[:, C * t : C * (t + 1)],
                            rhs=exps[:, CH * q : CH * (q + 1)],
                            start=(t == 0),
                            stop=(t == TJ - 1),
                        )
                return EPS

            # ------------- per-chunk scalar assembly -------------
            def assemble(k, EPS):
                i0 = k * CH
                # S row: colsum over classes of W2 * E
                Ecp = am.tile([C, CH], F32, name=f"Ecp{k}", tag="Ecp")
                nc.vector.tensor_copy(Ecp, EPS)
                W2E = am.tile([C, CH], F32, name=f"W2E{k}", tag="W2E")
                nc.vector.tensor_mul(W2E, Ecp, s_W2[:, i0 : i0 + CH])
                SrowPS = sp.tile([1, CH], F32, name=f"Srow{k}", tag="sm")
                nc.tensor.matmul(
                    SrowPS, lhsT=s_ones[0:C, :], rhs=W2E, start=True, stop=True
                )
                Srow = am.tile([1, CH], F32, name=f"Srowb{k}", tag="Srowb")
                nc.vector.tensor_copy(Srow, SrowPS)
                # A = gsum.T @ f ; Asel = A[lab_i, i]
                APS = sp.tile([C, CH], F32, name=f"APS{k}", tag="sm")
                nc.tensor.matmul(
                    APS,
                    lhsT=s_gsum,
                    rhs=s_fTc[:, i0 : i0 + CH],
                    start=True,
                    stop=True,
                )
                AtT = am.tile([C, CH], F32, name=f"AtT{k}", tag="AtT")
                nc.vector.tensor_mul(AtT, APS, s_tTp[:, i0 : i0 + CH])
                AselPS = sp.tile([1, CH], F32, name=f"Asel{k}", tag="sm")
                nc.tensor.matmul(
                    AselPS, lhsT=s_ones[0:C, :], rhs=AtT, start=True, stop=True
                )
                Asel = am.tile([1, CH], F32, name=f"Aselb{k}", tag="Aselb")
                nc.vector.tensor_copy(Asel, AselPS)
                # fsq = ||f_i||^2 from the same bf16 features
                sq = am.tile([128, CH], F32, name=f"sq{k}", tag="sq")
                nc.vector.tensor_mul(
                    sq, s_fTc[:, i0 : i0 + CH], s_fTc[:, i0 : i0 + CH]
                )
                fsqPS = sp.tile([1, CH], F32, name=f"fsq{k}", tag="sm")
                nc.tensor.matmul(fsqPS, lhsT=s_ones, rhs=sq, start=True, stop=True)
                # ed = exp(10*fsq) requantized to bf16 to match E's stored exp
                ed = am.tile([1, CH], F32, name=f"ed{k}", tag="ed")
                nc.scalar.activation(out=ed, in_=fsqPS, func=AF.Exp, scale=1.0 / TEMP)
                ed_bf = am.tile([1, CH], BF16, name=f"edb{k}", tag="edb")
                nc.vector.tensor_copy(ed_bf, ed)
                ed_q = am.tile([1, CH], F32, name=f"edq{k}", tag="edq")
                nc.vector.tensor_copy(ed_q, ed_bf)
                fsq_sb = am.tile([1, CH], F32, name=f"fsqb{k}", tag="fsqb")
                nc.vector.tensor_copy(fsq_sb, fsqPS)
                # m/10 then reciprocal -> 10/m
                mPS = sp.tile([1, CH], F32, name=f"m{k}", tag="sm")
                nc.tensor.matmul(
                    mPS, lhsT=s_ccm1, rhs=s_tTp[:, i0 : i0 + CH], start=True, stop=True
                )
                minv10 = am.tile([1, CH], F32, name=f"minv{k}", tag="minv")
                nc.vector.reciprocal(minv10, mPS)
                # S -= ed_q / m  (= ed_q * minv10 * 0.1)
                dg = am.tile([1, CH], F32, name=f"dg{k}", tag="dg")
                nc.vector.tensor_mul(dg, ed_q, minv10)
                dg2 = am.tile([1, CH], F32, name=f"dg2{k}", tag="dg2")
                nc.vector.tensor_scalar_mul(dg2, dg, 0.1)
                nc.vector.tensor_sub(s_Scorr[:, i0 : i0 + CH], Srow, dg2)
                # SmT = (Asel - fsq) * 10/m
                smr = am.tile([1, CH], F32, name=f"smr{k}", tag="smr")
                nc.vector.tensor_sub(smr, Asel, fsq_sb)
                nc.vector.tensor_mul(s_SmT[:, i0 : i0 + CH], smr, minv10)

            # ---------------- schedule ----------------
            EPS0 = chunk_body(0)

            # gsum[:, c] = sum_j feats_all[j, :] one-hot-weighted (PE accum)
            gsumPS = sp.tile([D, C], F32, name="gsumPS", tag="sm")
            for t in range(TJ):
                nc.tensor.matmul(
                    gsumPS,
                    lhsT=s_fAn[:, 128 * t : 128 * (t + 1)],
                    rhs=s_TAg[:, C * t : C * (t + 1)],
                    start=(t == 0),
                    stop=(t == TJ - 1),
                )
            nc.vector.tensor_copy(s_gsum, gsumPS)

            EPS1 = chunk_body(1)
            assemble(0, EPS0)
            assemble(1, EPS1)

            # ---------------- final scalar reduction ----------------
            lg = am.tile([1, R], F32)
            nc.scalar.activation(out=lg, in_=s_Scorr, func=AF.Ln)
            lossrow = am.tile([1, R], F32)
            nc.vector.tensor_sub(lossrow, lg, s_SmT)
            wrow = am.tile([1, R], F32)
            nc.vector.tensor_mul(wrow, lossrow, s_conf)
            numv = am.tile([1, 1], F32)
            nc.vector.reduce_sum(out=numv, in_=wrow, axis=mybir.AxisListType.X)
            denv = am.tile([1, 1], F32)
            nc.vector.reduce_sum(out=denv, in_=s_conf, axis=mybir.AxisListType.X)
            outsb = am.tile([1, 2], F32)
            nc.vector.tensor_copy(outsb[:, 0:1], numv)
            nc.vector.tensor_copy(outsb[:, 1:2], denv)
            nc.sync.dma_start(out=outd[:], in_=outsb)

    nc.finalize()
    return nc


def _get_nc():
    if "nc" not in _NC_CACHE:
        _NC_CACHE["nc"] = _build_nc()
    return _NC_CACHE["nc"]


def _prep_inputs(centers1, features, targets, conf_mask):
    f32 = np.float32
    features = np.ascontiguousarray(features, dtype=f32)
    centers1 = np.ascontiguousarray(centers1, dtype=f32).reshape(-1, D)
    targets = np.ascontiguousarray(targets, dtype=f32)
    conf_mask = np.ascontiguousarray(conf_mask, dtype=f32)

    feats_all = np.concatenate([features, centers1], axis=0)  # [N, D]
    fa_pad = np.zeros((NPAD, D), dtype=f32)
    fa_pad[:N] = feats_all
    TA = np.concatenate([targets, np.eye(C, dtype=f32)], axis=0)  # [N, C]
    TA_pad = np.zeros((NPAD, C), dtype=f32)
    TA_pad[:N] = TA

    fTg_np = np.ascontiguousarray(fa_pad.T).astype(BF)  # [D, NPAD]
    fAn_np = np.ascontiguousarray(
        fa_pad.reshape(TJ, 128, D).transpose(1, 0, 2).reshape(128, TJ * D)
    ).astype(BF)
    TAg_np = np.ascontiguousarray(
        TA_pad.reshape(TJ, 128, C).transpose(1, 0, 2).reshape(128, TJ * C)
    ).astype(BF)

    cc = targets.sum(axis=0, dtype=np.float64) + 1.0  # [C]
    dcls = np.where(cc > 1.5, 1.0 / np.maximum(cc - 1.0, 1.0) - 1.0 / cc, 0.0)
    invc = 1.0 / cc
    ccm1_np = ((cc - 1.0) / 10.0).astype(f32).reshape(C, 1)

    in_maps = []
    for c in range(CORES):
        rows = slice(c * R, (c + 1) * R)
        fTc_np = np.ascontiguousarray(fTg_np[:, c * R : (c + 1) * R])
        tTp_np = np.ascontiguousarray(targets[rows].T, dtype=f32)  # [C, R]
        W2_np = (dcls[:, None] * tTp_np + invc[:, None]).astype(f32)
        conf_np = np.ascontiguousarray(conf_mask[rows].reshape(1, R), dtype=f32)
        in_maps.append(
            {
                "fTg": fTg_np,
                "fAn": fAn_np,
                "TAg": TAg_np,
                "fTc": fTc_np,
                "tTp": tTp_np,
                "W2": W2_np,
                "conf": conf_np,
                "ccm1": ccm1_np,
            }
        )
    return in_maps


def _run(centers1, features, targets, conf_mask, trace=False, trace_cores=None):
    in_maps = _prep_inputs(centers1, features, targets, conf_mask)
    nc = _get_nc()
    kwargs = {}
    if trace:
        # NTFF profiling under axon: shim the (absent) antenv.axon_hooks
        # module and skip the artifact bucket upload.
        import types
        import concourse.bass_utils as bass_utils

        if "antenv.axon_hooks" not in sys.modules:
            mod = types.ModuleType("antenv.axon_hooks")
            mod._hook = None

            def set_axon_ntff_profile_hook(h):
                mod._hook = h

            def get_axon_ntff_profile_hook():
                return mod._hook

            mod.set_axon_ntff_profile_hook = set_axon_ntff_profile_hook
            mod.get_axon_ntff_profile_hook = get_axon_ntff_profile_hook
            sys.modules["antenv.axon_hooks"] = mod
            from trn_agent_boot.trn_boot import _ntff_profile_via_ctypes

            set_axon_ntff_profile_hook(
                _ntff_profile_via_ctypes("/opt/axon/libaxon_pjrt.so")
            )
        bass_utils.upload_artifacts = lambda tmpdir: "local://" + tmpdir
        kwargs = {"trace": True}
        if trace_cores is not None:
            kwargs["trace_cores"] = trace_cores
    res = run_bass_kernel_spmd(nc, in_maps, core_ids=list(range(CORES)), **kwargs)
    num = 0.0
    den = 0.0
    for r in res.results:
        num += float(r["out"][0, 0])
        den += float(r["out"][0, 1])
    loss = np.array(num / den, dtype=np.float32)
    return loss, res


def kernel(centers1, features, targets, cls_num_list, conf_mask):
    loss, _ = _run(centers1, features, targets, conf_mask)
    return loss


# revision 5
# speedup vs baseline: 1.1276x; 1.1276x over previous
"""Trainium2 Bass kernel for the BalSCL/SSL balanced supervised-contrastive loss.

Distribution: data-parallel over the 8192 anchor rows, 1024 rows per core on
8 NeuronCores.  Each core computes a partial loss numerator / denominator and
the host combines the 8 scalar pairs.

Math (restructured from the reference, analytically identical):
  N = 8292 columns (8192 anchors + 100 class centers), all unit-norm.
  The row-max subtraction in the reference cancels analytically, so
    loss_i = log(S_i) - (10/m_i) * Sm_i
  with
    S_i  = sum_{j != i} exp(10 * f_i . g_j) / (cc_j - [lab_j == lab_i])
    Sm_i = sum_{j != i, lab_j == lab_i} f_i . g_j
    m_i  = cc[lab_i] - 1      (number of positive pairs for row i)
  Using the one-hot structure everything reduces to per-class aggregates on
  the tensor engine:
    E[c, i]   = sum_{j in class c} exp(10 * rawT[j, i])     (incl. j == i)
    gsum[c,:] = sum_{j in class c} g_j ;  gath[:, i] = gsum[lab_i, :]
  and the diagonal (j == i) contribution is subtracted analytically using
  ||f_i||^2, re-quantized to bf16 so it matches the bf16-stored exp that
  entered E bit-for-bit.  Per-row gathers over classes are one-hot matmuls,
  1/m comes from a per-class constant vector (no reciprocal needed).
"""

import sys

sys.path.insert(0, "/opt/trn_rl_repo")

import numpy as np
import ml_dtypes

import concourse.bass as bass  # noqa: F401
import concourse.bacc as bacc
import concourse.tile as tile
from concourse import mybir
from concourse.bass_utils import run_bass_kernel_spmd

F32 = mybir.dt.float32
BF16 = mybir.dt.bfloat16
BF = ml_dtypes.bfloat16
AF = mybir.ActivationFunctionType
ALU = mybir.AluOpType

B2, C, D = 8192, 100, 128
TEMP = 0.1
N = B2 + C                # 8292
TJ = (N + 127) // 128     # 65 j-tiles
NPAD = TJ * 128           # 8320
CORES = 8
R = B2 // CORES           # 1024 rows per core
CH = 512                  # i-chunk width (one fp32 PSUM bank)
NCH = R // CH             # 2 chunks per core
GW = 3                    # j-tiles per exp group (3 PSUM banks)
GROUPS = [(g * GW, min(GW, TJ - g * GW)) for g in range((TJ + GW - 1) // GW)]
N_WARM = 8                # PE warm-up matmuls (HAM un-throttle)

import os
FLAG_WARM = os.environ.get("KB_WARM", "1") == "1"
FLAG_GSUM_IL = os.environ.get("KB_GSUM_IL", "1") == "1"
FLAG_STT = os.environ.get("KB_STT", "1") == "1"
FLAG_TTR = os.environ.get("KB_TTR", "0") == "1"  # tensor_tensor_reduce crashes HW - keep off
FLAG_MIX = os.environ.get("KB_MIX", "1") == "1"

_NC_CACHE = {}


def _build_nc():
    nc = bacc.Bacc()

    fTg = nc.dram_tensor("fTg", [D, NPAD], BF16, kind="ExternalInput")
    fAn = nc.dram_tensor("fAn", [128, TJ * 128], BF16, kind="ExternalInput")
    TAg = nc.dram_tensor("TAg", [128, TJ * C], BF16, kind="ExternalInput")
    fTc = nc.dram_tensor("fTc", [D, R], BF16, kind="ExternalInput")
    tTp = nc.dram_tensor("tTp", [C, R], F32, kind="ExternalInput")
    W2 = nc.dram_tensor("W2", [C, R], F32, kind="ExternalInput")
    conf = nc.dram_tensor("conf", [1, R], F32, kind="ExternalInput")
    rcc = nc.dram_tensor("rcc", [C, 1], F32, kind="ExternalInput")
    outd = nc.dram_tensor("out", [1, 2], F32, kind="ExternalOutput")

    with tile.TileContext(nc) as tc:
        with (
            tc.tile_pool(name="consts", bufs=1) as cp,
            tc.tile_pool(name="expp", bufs=3) as ep,
            tc.tile_pool(name="asmp", bufs=2) as am,
            tc.tile_pool(name="rawp", bufs=2, space="PSUM") as rp,
            tc.tile_pool(name="epsp", bufs=1, space="PSUM") as pp,
            tc.tile_pool(name="smp", bufs=1, space="PSUM") as sp,
        ):
            # ---------------- input loads (ordered by first use) ----------
            s_fTc = cp.tile([D, R], BF16)
            nc.sync.dma_start(out=s_fTc, in_=fTc[:])
            s_fTg = cp.tile([D, NPAD], BF16)
            edges = [0, 3, 16, 29, 42, 55, TJ]
            for a, b in zip(edges, edges[1:]):
                nc.sync.dma_start(
                    out=s_fTg[:, a * 128 : b * 128], in_=fTg[:, a * 128 : b * 128]
                )
            s_TAg = cp.tile([128, TJ * C], BF16)
            tedges = [0, 6, 26, 46, TJ]
            for a, b in zip(tedges, tedges[1:]):
                nc.sync.dma_start(out=s_TAg[:, a * C : b * C], in_=TAg[:, a * C : b * C])
            s_fAn = cp.tile([128, TJ * 128], BF16)
            for i in range(4):
                c0 = i * 17 * 128
                c1 = min(TJ * 128, c0 + 17 * 128)
                nc.sync.dma_start(out=s_fAn[:, c0:c1], in_=fAn[:, c0:c1])
            s_tTp = cp.tile([C, R], F32)
            nc.sync.dma_start(out=s_tTp, in_=tTp[:])
            s_W2 = cp.tile([C, R], F32)
            nc.sync.dma_start(out=s_W2, in_=W2[:])
            s_conf = cp.tile([1, R], F32)
            nc.sync.dma_start(out=s_conf, in_=conf[:])
            s_rcc = cp.tile([C, 1], F32)
            nc.sync.dma_start(out=s_rcc, in_=rcc[:])

            s_ones = cp.tile([128, 1], F32)
            nc.vector.memset(s_ones, 1.0)
            s_nones = cp.tile([128, 1], F32)
            nc.vector.memset(s_nones, -1.0)
            s_scr = cp.tile([128, CH], BF16)
            nc.gpsimd.memset(s_scr, 1.0)

            s_gsum = cp.tile([C, D], F32)       # gsum[c, d] (fp32)
            s_Scorr = cp.tile([1, R], F32)
            s_SmT = cp.tile([1, R], F32)

            # ---- PE warm-up: dense junk matmuls so HAM un-throttles early
            if FLAG_WARM:
                warmPS = sp.tile([128, CH], F32, name="warmPS", tag="sm")
                for _ in range(N_WARM):
                    nc.tensor.matmul(
                        warmPS, lhsT=s_scr[:, 0:128], rhs=s_scr, start=True, stop=True
                    )

            # conf denominator (off the critical tail)
            denv = am.tile([1, 1], F32)
            nc.vector.reduce_sum(out=denv, in_=s_conf, axis=mybir.AxisListType.X)

            # ------------- per-chunk raw/exp/E pipeline -------------
            # `extras` is a list of closures; one is emitted after each group
            # so the scalar-assembly smalls interleave with the main stream.
            def chunk_body(k, extras=()):
                i0 = k * CH
                extras = list(extras)
                EPS = pp.tile([C, CH], F32, name=f"EPS{k}", tag="EPS")
                for gi, (t0, gw) in enumerate(GROUPS):
                    rawPS = rp.tile([128, CH * GW], F32, name="rawPS", tag="raw")
                    for q in range(gw):
                        t = t0 + q
                        nc.tensor.matmul(
                            rawPS[:, CH * q : CH * (q + 1)],
                            lhsT=s_fTg[:, 128 * t : 128 * (t + 1)],
                            rhs=s_fTc[:, i0 : i0 + CH],
                            start=True,
                            stop=True,
                        )
                    exps = ep.tile([128, CH * GW], BF16, name="exps", tag="exps")
                    nc.scalar.activation(
                        out=exps[:, : CH * gw],
                        in_=rawPS[:, : CH * gw],
                        func=AF.Exp,
                        scale=1.0 / TEMP,
                    )
                    for q in range(gw):
                        t = t0 + q
                        nc.tensor.matmul(
                            EPS,
                            lhsT=s_TAg[:, C * t : C * (t + 1)],
                            rhs=exps[:, CH * q : CH * (q + 1)],
                            start=(t == 0),
                            stop=(t == TJ - 1),
                        )
                    if extras and gi >= 1:
                        extras.pop(0)()
                for fn in extras:
                    fn()
                return EPS

            # ---------------- chunk 0 + interleaved gsum ----------------
            # gsum[c, :] = sum_{j in class c} feats_all[j, :]; accumulated in
            # PSUM across the 65 j-tiles, interleaved into chunk 0's groups.
            gsumPS = sp.tile([C, D], F32, name="gsumPS", tag="sm")
            gsum_state = {"t": 0}

            def gsum_step():
                t0 = gsum_state["t"]
                for t in range(t0, min(t0 + 4, TJ)):
                    nc.tensor.matmul(
                        gsumPS,
                        lhsT=s_TAg[:, C * t : C * (t + 1)],
                        rhs=s_fAn[:, 128 * t : 128 * (t + 1)],
                        start=(t == 0),
                        stop=(t == TJ - 1),
                    )
                gsum_state["t"] = min(t0 + 4, TJ)

            if FLAG_GSUM_IL:
                EPS0 = chunk_body(0, extras=[gsum_step] * ((TJ + 3) // 4))
            else:
                EPS0 = chunk_body(0)
                while gsum_state["t"] < TJ:
                    gsum_step()
            nc.vector.tensor_copy(s_gsum, gsumPS)

            # ---- free EPS0 early so chunk 1's accumulation can begin
            W2E0 = am.tile([C, CH], F32, name="W2E0", tag="W2E")
            nc.vector.tensor_mul(W2E0, EPS0, s_W2[:, 0:CH])

            # ------------- scalar assembly pieces (closures) -------------
            sq_t = [None, None]
            f32c_t = [None, None]
            minv_t = [None, None]
            dg_t = [None, None]
            gmul_t = [None, None]

            def mk_m(k):
                def go():
                    i0 = k * CH
                    mPS = sp.tile([1, CH], F32, name=f"mPS{k}", tag="sm")
                    nc.tensor.matmul(
                        mPS, lhsT=s_rcc, rhs=s_tTp[:, i0 : i0 + CH],
                        start=True, stop=True,
                    )
                    minv = am.tile([1, CH], F32, name=f"minv{k}", tag="minv")
                    nc.vector.tensor_copy(minv, mPS)
                    minv_t[k] = minv
                return go

            def mk_fsq(k):
                def go():
                    i0 = k * CH
                    sq = am.tile([128, CH], F32, name=f"sq{k}", tag="sq")
                    if FLAG_MIX:
                        nc.vector.tensor_mul(
                            sq, s_fTc[:, i0 : i0 + CH], s_fTc[:, i0 : i0 + CH]
                        )
                    else:
                        f32c = am.tile([128, CH], F32, name=f"f32c{k}", tag="f32c")
                        nc.vector.tensor_copy(f32c, s_fTc[:, i0 : i0 + CH])
                        f32c_t[k] = f32c
                        nc.vector.tensor_mul(sq, f32c, f32c)
                    sq_t[k] = sq
                    fsqPS = sp.tile([1, CH], F32, name=f"fsqPS{k}", tag="sm")
                    nc.tensor.matmul(fsqPS, lhsT=s_ones, rhs=sq, start=True, stop=True)
                    ed_bf = am.tile([1, CH], BF16, name=f"edb{k}", tag="edb")
                    nc.scalar.activation(
                        out=ed_bf, in_=fsqPS, func=AF.Exp, scale=1.0 / TEMP
                    )
                    # dg = (ed * 0.1) * (10/m)  = exp(10 fsq) / m
                    dg = am.tile([1, CH], F32, name=f"dg{k}", tag="dg")
                    if FLAG_STT:
                        nc.vector.scalar_tensor_tensor(
                            out=dg, in0=ed_bf, scalar=0.1, in1=minv_t[k],
                            op0=ALU.mult, op1=ALU.mult,
                        )
                    else:
                        ed_q = am.tile([1, CH], F32, name=f"edq{k}", tag="edq")
                        nc.vector.tensor_copy(ed_q, ed_bf)
                        dga = am.tile([1, CH], F32, name=f"dga{k}", tag="dga")
                        nc.vector.tensor_mul(dga, ed_q, minv_t[k])
                        nc.vector.tensor_scalar_mul(dg, dga, 0.1)
                    dg_t[k] = dg
                return go

            def mk_gath(k):
                def go():
                    i0 = k * CH
                    gathT = sp.tile([D, CH], F32, name=f"gathT{k}", tag="sm")
                    nc.tensor.matmul(
                        gathT, lhsT=s_gsum, rhs=s_tTp[:, i0 : i0 + CH],
                        start=True, stop=True,
                    )
                    gmul = am.tile([128, CH], F32, name=f"gmul{k}", tag="gmul")
                    if FLAG_MIX:
                        nc.vector.tensor_mul(gmul, gathT, s_fTc[:, i0 : i0 + CH])
                    else:
                        nc.vector.tensor_mul(gmul, gathT, f32c_t[k])
                    gmul_t[k] = gmul
                return go

            def mk_smr(k):
                def go():
                    i0 = k * CH
                    smrPS = sp.tile([1, CH], F32, name=f"smrPS{k}", tag="sm")
                    nc.tensor.matmul(
                        smrPS, lhsT=s_ones, rhs=gmul_t[k], start=True, stop=False
                    )
                    nc.tensor.matmul(
                        smrPS, lhsT=s_nones, rhs=sq_t[k], start=False, stop=True
                    )
                    nc.vector.tensor_mul(
                        s_SmT[:, i0 : i0 + CH], smrPS, minv_t[k]
                    )
                return go

            def mk_srow(k, W2E):
                def go():
                    i0 = k * CH
                    SrowPS = sp.tile([1, CH], F32, name=f"SrowPS{k}", tag="sm")
                    nc.tensor.matmul(
                        SrowPS, lhsT=s_ones[0:C, :], rhs=W2E, start=True, stop=True
                    )
                    nc.vector.tensor_sub(
                        s_Scorr[:, i0 : i0 + CH], SrowPS, dg_t[k]
                    )
                return go

            closures = [
                mk_m(0), mk_fsq(0), mk_m(1), mk_fsq(1),
                mk_gath(0), mk_smr(0), mk_gath(1), mk_smr(1),
                mk_srow(0, W2E0),
            ]
            EPS1 = chunk_body(1, extras=closures)

            # ---------------- tail ----------------
            W2E1 = am.tile([C, CH], F32, name="W2E1", tag="W2E")
            nc.vector.tensor_mul(W2E1, EPS1, s_W2[:, CH : 2 * CH])
            mk_srow(1, W2E1)()

            lg = am.tile([1, R], F32)
            nc.scalar.activation(out=lg, in_=s_Scorr, func=AF.Ln)
            diff = am.tile([1, R], F32)
            nc.vector.tensor_sub(diff, lg, s_SmT)
            wrow = am.tile([1, R], F32)
            numv = am.tile([1, 1], F32)
            if FLAG_TTR:
                nc.vector.tensor_tensor_reduce(
                    out=wrow, in0=diff, in1=s_conf, scale=1.0, scalar=0.0,
                    op0=ALU.mult, op1=ALU.add, accum_out=numv,
                )
            else:
                nc.vector.tensor_mul(wrow, diff, s_conf)
                nc.vector.reduce_sum(out=numv, in_=wrow, axis=mybir.AxisListType.X)
            outsb = am.tile([1, 2], F32)
            nc.vector.tensor_copy(outsb[:, 0:1], numv)
            nc.vector.tensor_copy(outsb[:, 1:2], denv)
            nc.sync.dma_start(out=outd[:], in_=outsb)

    nc.finalize()
    return nc


def _get_nc():
    if "nc" not in _NC_CACHE:
        _NC_CACHE["nc"] = _build_nc()
    return _NC_CACHE["nc"]


def _prep_inputs(centers1, features, targets, conf_mask):
    f32 = np.float32
    features = np.ascontiguousarray(features, dtype=f32)
    centers1 = np.ascontiguousarray(centers1, dtype=f32).reshape(-1, D)
    targets = np.ascontiguousarray(targets, dtype=f32)
    conf_mask = np.ascontiguousarray(conf_mask, dtype=f32)

    feats_all = np.concatenate([features, centers1], axis=0)  # [N, D]
    fa_pad = np.zeros((NPAD, D), dtype=f32)
    fa_pad[:N] = feats_all
    TA = np.concatenate([targets, np.eye(C, dtype=f32)], axis=0)  # [N, C]
    TA_pad = np.zeros((NPAD, C), dtype=f32)
    TA_pad[:N] = TA

    fTg_np = np.ascontiguousarray(fa_pad.T).astype(BF)  # [D, NPAD]
    fAn_np = np.ascontiguousarray(
        fa_pad.reshape(TJ, 128, D).transpose(1, 0, 2).reshape(128, TJ * D)
    ).astype(BF)
    TAg_np = np.ascontiguousarray(
        TA_pad.reshape(TJ, 128, C).transpose(1, 0, 2).reshape(128, TJ * C)
    ).astype(BF)

    cc = targets.sum(axis=0, dtype=np.float64) + 1.0  # [C]
    safe = cc > 1.5
    dcls = np.where(safe, 1.0 / np.maximum(cc - 1.0, 1.0) - 1.0 / cc, 0.0)
    invc = 1.0 / cc
    rcc_np = np.where(safe, 10.0 / np.maximum(cc - 1.0, 1.0), 0.0)
    rcc_np = rcc_np.astype(f32).reshape(C, 1)

    in_maps = []
    for c in range(CORES):
        rows = slice(c * R, (c + 1) * R)
        fTc_np = np.ascontiguousarray(fTg_np[:, c * R : (c + 1) * R])
        tTp_np = np.ascontiguousarray(targets[rows].T, dtype=f32)  # [C, R]
        W2_np = (dcls[:, None] * tTp_np + invc[:, None]).astype(f32)
        conf_np = np.ascontiguousarray(conf_mask[rows].reshape(1, R), dtype=f32)
        in_maps.append(
            {
                "fTg": fTg_np,
                "fAn": fAn_np,
                "TAg": TAg_np,
                "fTc": fTc_np,
                "tTp": tTp_np,
                "W2": W2_np,
                "conf": conf_np,
                "rcc": rcc_np,
            }
        )
    return in_maps


def _run(centers1, features, targets, conf_mask, trace=False, trace_cores=None):
    in_maps = _prep_inputs(centers1, features, targets, conf_mask)
    nc = _get_nc()
    kwargs = {}
    if trace:
        # NTFF profiling under axon: shim the (absent) antenv.axon_hooks
        # module and skip the artifact bucket upload.
        import types
        import concourse.bass_utils as bass_utils

        if "antenv.axon_hooks" not in sys.modules:
            mod = types.ModuleType("antenv.axon_hooks")
            mod._hook = None

            def set_axon_ntff_profile_hook(h):
                mod._hook = h

            def get_axon_ntff_profile_hook():
                return mod._hook

            mod.set_axon_ntff_profile_hook = set_axon_ntff_profile_hook
            mod.get_axon_ntff_profile_hook = get_axon_ntff_profile_hook
            sys.modules["antenv.axon_hooks"] = mod
            from trn_agent_boot.trn_boot import _ntff_profile_via_ctypes

            set_axon_ntff_profile_hook(
                _ntff_profile_via_ctypes("/opt/axon/libaxon_pjrt.so")
            )
        bass_utils.upload_artifacts = lambda tmpdir: "local://" + tmpdir
        kwargs = {"trace": True}
        if trace_cores is not None:
            kwargs["trace_cores"] = trace_cores
    res = run_bass_kernel_spmd(nc, in_maps, core_ids=list(range(CORES)), **kwargs)
    num = 0.0
    den = 0.0
    for r in res.results:
        num += float(r["out"][0, 0])
        den += float(r["out"][0, 1])
    loss = np.array(num / den, dtype=np.float32)
    return loss, res


def kernel(centers1, features, targets, cls_num_list, conf_mask):
    loss, _ = _run(centers1, features, targets, conf_mask)
    return loss


# revision 6
# speedup vs baseline: 1.1663x; 1.0343x over previous
"""Trainium2 Bass kernel for the BalSCL/SSL balanced supervised-contrastive loss.

Distribution: data-parallel over the 8192 anchor rows, 1024 rows per core on
8 NeuronCores.  Each core computes a partial loss numerator / denominator and
the host combines the 8 scalar pairs.

Math (restructured from the reference, analytically identical):
  N = 8292 columns (8192 anchors + 100 class centers), all unit-norm.
  The row-max subtraction in the reference cancels analytically, so
    loss_i = log(S_i) - (10/m_i) * Sm_i
  with
    S_i  = sum_{j != i} exp(10 * f_i . g_j) / (cc_j - [lab_j == lab_i])
    Sm_i = sum_{j != i, lab_j == lab_i} f_i . g_j
    m_i  = cc[lab_i] - 1      (number of positive pairs for row i)
  Using the one-hot structure everything reduces to per-class aggregates on
  the tensor engine:
    E[c, i]   = sum_{j in class c} exp(10 * rawT[j, i])     (incl. j == i)
    gsum[c,:] = sum_{j in class c} g_j ;  gath[:, i] = gsum[lab_i, :]
  and the diagonal (j == i) contribution is subtracted analytically using
  ||f_i||^2, re-quantized to bf16 so it matches the bf16-stored exp that
  entered E bit-for-bit.  Per-row gathers over classes are one-hot matmuls,
  1/m comes from a per-class constant vector (no reciprocal needed).
"""

import sys

sys.path.insert(0, "/opt/trn_rl_repo")

import numpy as np
import ml_dtypes

import concourse.bass as bass  # noqa: F401
import concourse.bacc as bacc
import concourse.tile as tile
from concourse import mybir
from concourse.bass_utils import run_bass_kernel_spmd

F32 = mybir.dt.float32
BF16 = mybir.dt.bfloat16
BF = ml_dtypes.bfloat16
AF = mybir.ActivationFunctionType
ALU = mybir.AluOpType

B2, C, D = 8192, 100, 128
TEMP = 0.1
N = B2 + C                # 8292
TJ = (N + 127) // 128     # 65 j-tiles
NPAD = TJ * 128           # 8320
CORES = 8
R = B2 // CORES           # 1024 rows per core
CH = 512                  # i-chunk width (one fp32 PSUM bank)
NCH = R // CH             # 2 chunks per core
GW = 3                    # j-tiles per exp group (3 PSUM banks)
GROUPS = [(g * GW, min(GW, TJ - g * GW)) for g in range((TJ + GW - 1) // GW)]
N_WARM = 10               # PE warm-up matmuls (HAM un-throttle)

import os
FLAG_WARM = os.environ.get("KB_WARM", "1") == "1"
FLAG_GSUM_IL = os.environ.get("KB_GSUM_IL", "1") == "1"
FLAG_STT = os.environ.get("KB_STT", "1") == "1"
FLAG_TTR = os.environ.get("KB_TTR", "0") == "1"  # tensor_tensor_reduce crashes HW - keep off
FLAG_MIX = os.environ.get("KB_MIX", "1") == "1"

_NC_CACHE = {}


def _build_nc():
    nc = bacc.Bacc()

    fTg = nc.dram_tensor("fTg", [D, NPAD], BF16, kind="ExternalInput")
    fAn = nc.dram_tensor("fAn", [128, TJ * 128], BF16, kind="ExternalInput")
    TAg = nc.dram_tensor("TAg", [128, TJ * C], BF16, kind="ExternalInput")
    fTc = nc.dram_tensor("fTc", [D, R], BF16, kind="ExternalInput")
    tTp = nc.dram_tensor("tTp", [C, R], BF16, kind="ExternalInput")
    W2 = nc.dram_tensor("W2", [C, R], F32, kind="ExternalInput")
    conf = nc.dram_tensor("conf", [1, R], F32, kind="ExternalInput")
    rcc = nc.dram_tensor("rcc", [C, 1], BF16, kind="ExternalInput")
    outd = nc.dram_tensor("out", [1, 2], F32, kind="ExternalOutput")

    with tile.TileContext(nc) as tc:
        with (
            tc.tile_pool(name="consts", bufs=1) as cp,
            tc.tile_pool(name="expp", bufs=3) as ep,
            tc.tile_pool(name="asmp", bufs=2) as am,
            tc.tile_pool(name="rawp", bufs=2, space="PSUM") as rp,
            tc.tile_pool(name="epsp", bufs=1, space="PSUM") as pp,
            tc.tile_pool(name="smp", bufs=1, space="PSUM") as sp,
        ):
            # ---------------- input loads (ordered by first use) ----------
            s_fTc = cp.tile([D, R], BF16)
            nc.sync.dma_start(out=s_fTc, in_=fTc[:])
            s_fTg = cp.tile([D, NPAD], BF16)
            edges = [0, 2, 15, 28, 41, 54, TJ]
            for a, b in zip(edges, edges[1:]):
                nc.sync.dma_start(
                    out=s_fTg[:, a * 128 : b * 128], in_=fTg[:, a * 128 : b * 128]
                )
            s_TAg = cp.tile([128, TJ * C], BF16)
            tedges = [0, 6, 26, 46, TJ]
            for a, b in zip(tedges, tedges[1:]):
                nc.sync.dma_start(out=s_TAg[:, a * C : b * C], in_=TAg[:, a * C : b * C])
            s_fAn = cp.tile([128, TJ * 128], BF16)
            for i in range(4):
                c0 = i * 17 * 128
                c1 = min(TJ * 128, c0 + 17 * 128)
                nc.sync.dma_start(out=s_fAn[:, c0:c1], in_=fAn[:, c0:c1])
            s_tTp = cp.tile([C, R], BF16)
            nc.sync.dma_start(out=s_tTp, in_=tTp[:])
            s_W2 = cp.tile([C, R], F32)
            nc.sync.dma_start(out=s_W2, in_=W2[:])
            s_conf = cp.tile([1, R], F32)
            nc.sync.dma_start(out=s_conf, in_=conf[:])
            s_rcc = cp.tile([C, 1], BF16)
            nc.sync.dma_start(out=s_rcc, in_=rcc[:])

            s_ones = cp.tile([128, 1], F32)
            nc.vector.memset(s_ones, 1.0)
            s_nones = cp.tile([128, 1], F32)
            nc.vector.memset(s_nones, -1.0)
            s_ones_bf = cp.tile([128, 1], BF16)
            nc.vector.memset(s_ones_bf, 1.0)
            s_scr = cp.tile([128, CH], BF16)
            nc.gpsimd.memset(s_scr, 1.0)

            s_gsum = cp.tile([C, D], BF16)      # gsum[c, d]
            s_Scorr = cp.tile([1, R], F32)
            s_SmT = cp.tile([1, R], F32)

            # ---- PE warm-up: dense junk matmuls so HAM un-throttles early
            if FLAG_WARM:
                warmPS = sp.tile([128, CH], F32, name="warmPS", tag="sm")
                for _ in range(N_WARM):
                    nc.tensor.matmul(
                        warmPS, lhsT=s_scr[:, 0:128], rhs=s_scr, start=True, stop=True
                    )

            # conf denominator (off the critical tail)
            denv = am.tile([1, 1], F32)
            nc.vector.reduce_sum(out=denv, in_=s_conf, axis=mybir.AxisListType.X)

            # ------------- per-chunk raw/exp/E pipeline -------------
            # `extras` is a list of closures; one is emitted after each group
            # so the scalar-assembly smalls interleave with the main stream.
            def chunk_body(k, extras=()):
                i0 = k * CH
                extras = list(extras)
                EPS = pp.tile([C, CH], F32, name=f"EPS{k}", tag="EPS")
                for gi, (t0, gw) in enumerate(GROUPS):
                    rawPS = rp.tile([128, CH * GW], F32, name="rawPS", tag="raw")
                    for q in range(gw):
                        t = t0 + q
                        nc.tensor.matmul(
                            rawPS[:, CH * q : CH * (q + 1)],
                            lhsT=s_fTg[:, 128 * t : 128 * (t + 1)],
                            rhs=s_fTc[:, i0 : i0 + CH],
                            start=True,
                            stop=True,
                        )
                    exps = ep.tile([128, CH * GW], BF16, name="exps", tag="exps")
                    nc.scalar.activation(
                        out=exps[:, : CH * gw],
                        in_=rawPS[:, : CH * gw],
                        func=AF.Exp,
                        scale=1.0 / TEMP,
                    )
                    for q in range(gw):
                        t = t0 + q
                        nc.tensor.matmul(
                            EPS,
                            lhsT=s_TAg[:, C * t : C * (t + 1)],
                            rhs=exps[:, CH * q : CH * (q + 1)],
                            start=(t == 0),
                            stop=(t == TJ - 1),
                        )
                    if extras and gi >= 1:
                        extras.pop(0)()
                for fn in extras:
                    fn()
                return EPS

            # ---------------- chunk 0 + interleaved gsum ----------------
            # gsum[c, :] = sum_{j in class c} feats_all[j, :]; accumulated in
            # PSUM across the 65 j-tiles, interleaved into chunk 0's groups.
            gsumPS = sp.tile([C, D], F32, name="gsumPS", tag="sm")
            gsum_state = {"t": 0}

            def gsum_step():
                t0 = gsum_state["t"]
                for t in range(t0, min(t0 + 4, TJ)):
                    nc.tensor.matmul(
                        gsumPS,
                        lhsT=s_TAg[:, C * t : C * (t + 1)],
                        rhs=s_fAn[:, 128 * t : 128 * (t + 1)],
                        start=(t == 0),
                        stop=(t == TJ - 1),
                    )
                gsum_state["t"] = min(t0 + 4, TJ)

            if FLAG_GSUM_IL:
                EPS0 = chunk_body(0, extras=[gsum_step] * ((TJ + 3) // 4))
            else:
                EPS0 = chunk_body(0)
                while gsum_state["t"] < TJ:
                    gsum_step()
            nc.vector.tensor_copy(s_gsum, gsumPS)

            # ---- free EPS0 early so chunk 1's accumulation can begin
            W2E0 = am.tile([C, CH], BF16, name="W2E0", tag="W2E")
            nc.vector.tensor_mul(W2E0, EPS0, s_W2[:, 0:CH])

            # ------------- scalar assembly pieces (closures) -------------
            sq_t = [None, None]
            f32c_t = [None, None]
            numB_t = [None, None]
            minv_t = [None, None]
            dg_t = [None, None]
            gmul_t = [None, None]

            def mk_m(k):
                def go():
                    i0 = k * CH
                    mPS = sp.tile([1, CH], F32, name=f"mPS{k}", tag="sm")
                    nc.tensor.matmul(
                        mPS, lhsT=s_rcc, rhs=s_tTp[:, i0 : i0 + CH],
                        start=True, stop=True,
                    )
                    minv = am.tile([1, CH], F32, name=f"minv{k}", tag="minv")
                    nc.vector.tensor_copy(minv, mPS)
                    minv_t[k] = minv
                return go

            def mk_fsq(k):
                def go():
                    i0 = k * CH
                    sq = am.tile([128, CH], F32, name=f"sq{k}", tag="sq")
                    if FLAG_MIX:
                        nc.vector.tensor_mul(
                            sq, s_fTc[:, i0 : i0 + CH], s_fTc[:, i0 : i0 + CH]
                        )
                    else:
                        f32c = am.tile([128, CH], F32, name=f"f32c{k}", tag="f32c")
                        nc.vector.tensor_copy(f32c, s_fTc[:, i0 : i0 + CH])
                        f32c_t[k] = f32c
                        nc.vector.tensor_mul(sq, f32c, f32c)
                    sq_t[k] = sq
                    fsqPS = sp.tile([1, CH], F32, name=f"fsqPS{k}", tag="sm")
                    nc.tensor.matmul(fsqPS, lhsT=s_ones, rhs=sq, start=True, stop=True)
                    ed_bf = am.tile([1, CH], BF16, name=f"edb{k}", tag="edb")
                    nc.scalar.activation(
                        out=ed_bf, in_=fsqPS, func=AF.Exp, scale=1.0 / TEMP
                    )
                    # dg = (ed * 0.1) * (10/m)  = exp(10 fsq) / m
                    dg = am.tile([1, CH], F32, name=f"dg{k}", tag="dg")
                    if FLAG_STT:
                        nc.vector.scalar_tensor_tensor(
                            out=dg, in0=ed_bf, scalar=0.1, in1=minv_t[k],
                            op0=ALU.mult, op1=ALU.mult,
                        )
                    else:
                        ed_q = am.tile([1, CH], F32, name=f"edq{k}", tag="edq")
                        nc.vector.tensor_copy(ed_q, ed_bf)
                        dga = am.tile([1, CH], F32, name=f"dga{k}", tag="dga")
                        nc.vector.tensor_mul(dga, ed_q, minv_t[k])
                        nc.vector.tensor_scalar_mul(dg, dga, 0.1)
                    dg_t[k] = dg
                return go

            def mk_gath(k):
                def go():
                    i0 = k * CH
                    gathT = sp.tile([D, CH], F32, name=f"gathT{k}", tag="sm")
                    nc.tensor.matmul(
                        gathT, lhsT=s_gsum, rhs=s_tTp[:, i0 : i0 + CH],
                        start=True, stop=True,
                    )
                    gmul = am.tile([128, CH], BF16, name=f"gmul{k}", tag="gmul")
                    if FLAG_MIX:
                        nc.vector.tensor_mul(gmul, gathT, s_fTc[:, i0 : i0 + CH])
                    else:
                        nc.vector.tensor_mul(gmul, gathT, f32c_t[k])
                    gmul_t[k] = gmul
                return go

            def mk_smr(k):
                def go():
                    i0 = k * CH
                    smrPS = sp.tile([1, CH], F32, name=f"smrPS{k}", tag="sm")
                    nc.tensor.matmul(
                        smrPS, lhsT=s_ones_bf, rhs=gmul_t[k], start=True, stop=False
                    )
                    nc.tensor.matmul(
                        smrPS, lhsT=s_nones, rhs=sq_t[k], start=False, stop=True
                    )
                    nc.vector.tensor_mul(
                        s_SmT[:, i0 : i0 + CH], smrPS, minv_t[k]
                    )
                return go

            def mk_smtc(k):
                def go():
                    i0 = k * CH
                    smtc = am.tile([1, CH], F32, name=f"smtc{k}", tag="smtc")
                    nc.vector.tensor_mul(
                        smtc, s_SmT[:, i0 : i0 + CH], s_conf[:, i0 : i0 + CH]
                    )
                    numB = am.tile([1, 1], F32, name=f"numB{k}", tag="numB")
                    nc.vector.reduce_sum(out=numB, in_=smtc, axis=mybir.AxisListType.X)
                    numB_t[k] = numB
                return go

            def mk_srow(k, W2E):
                def go():
                    i0 = k * CH
                    SrowPS = sp.tile([1, CH], F32, name=f"SrowPS{k}", tag="sm")
                    nc.tensor.matmul(
                        SrowPS, lhsT=s_ones_bf[0:C, :], rhs=W2E, start=True, stop=True
                    )
                    nc.vector.tensor_sub(
                        s_Scorr[:, i0 : i0 + CH], SrowPS, dg_t[k]
                    )
                return go

            closures = [
                mk_m(0), mk_fsq(0), mk_m(1), mk_fsq(1),
                mk_gath(0), mk_smr(0), mk_gath(1), mk_smr(1),
                mk_smtc(0), mk_smtc(1), mk_srow(0, W2E0),
            ]
            EPS1 = chunk_body(1, extras=closures)

            # ---------------- tail ----------------
            W2E1 = am.tile([C, CH], BF16, name="W2E1", tag="W2E")
            nc.vector.tensor_mul(W2E1, EPS1, s_W2[:, CH : 2 * CH])
            mk_srow(1, W2E1)()

            lg = am.tile([1, R], F32)
            nc.scalar.activation(out=lg, in_=s_Scorr, func=AF.Ln)
            wrow = am.tile([1, R], F32)
            nc.vector.tensor_mul(wrow, lg, s_conf)
            numA = am.tile([1, 1], F32)
            nc.vector.reduce_sum(out=numA, in_=wrow, axis=mybir.AxisListType.X)
            numAB = am.tile([1, 1], F32)
            nc.vector.tensor_sub(numAB, numA, numB_t[0])
            numv = am.tile([1, 1], F32)
            nc.vector.tensor_sub(numv, numAB, numB_t[1])
            outsb = am.tile([1, 2], F32)
            nc.vector.tensor_copy(outsb[:, 0:1], numv)
            nc.vector.tensor_copy(outsb[:, 1:2], denv)
            nc.sync.dma_start(out=outd[:], in_=outsb)

    nc.finalize()
    return nc


def _get_nc():
    if "nc" not in _NC_CACHE:
        _NC_CACHE["nc"] = _build_nc()
    return _NC_CACHE["nc"]


def _prep_inputs(centers1, features, targets, conf_mask):
    f32 = np.float32
    features = np.ascontiguousarray(features, dtype=f32)
    centers1 = np.ascontiguousarray(centers1, dtype=f32).reshape(-1, D)
    targets = np.ascontiguousarray(targets, dtype=f32)
    conf_mask = np.ascontiguousarray(conf_mask, dtype=f32)

    feats_all = np.concatenate([features, centers1], axis=0)  # [N, D]
    fa_pad = np.zeros((NPAD, D), dtype=f32)
    fa_pad[:N] = feats_all
    TA = np.concatenate([targets, np.eye(C, dtype=f32)], axis=0)  # [N, C]
    TA_pad = np.zeros((NPAD, C), dtype=f32)
    TA_pad[:N] = TA

    fTg_np = np.ascontiguousarray(fa_pad.T).astype(BF)  # [D, NPAD]
    fAn_np = np.ascontiguousarray(
        fa_pad.reshape(TJ, 128, D).transpose(1, 0, 2).reshape(128, TJ * D)
    ).astype(BF)
    TAg_np = np.ascontiguousarray(
        TA_pad.reshape(TJ, 128, C).transpose(1, 0, 2).reshape(128, TJ * C)
    ).astype(BF)

    cc = targets.sum(axis=0, dtype=np.float64) + 1.0  # [C]
    safe = cc > 1.5
    dcls = np.where(safe, 1.0 / np.maximum(cc - 1.0, 1.0) - 1.0 / cc, 0.0)
    invc = 1.0 / cc
    rcc_np = np.where(safe, 10.0 / np.maximum(cc - 1.0, 1.0), 0.0)
    rcc_np = rcc_np.astype(BF).reshape(C, 1)

    in_maps = []
    for c in range(CORES):
        rows = slice(c * R, (c + 1) * R)
        fTc_np = np.ascontiguousarray(fTg_np[:, c * R : (c + 1) * R])
        tTp_f32 = np.ascontiguousarray(targets[rows].T, dtype=f32)  # [C, R]
        tTp_np = tTp_f32.astype(BF)
        W2_np = (dcls[:, None] * tTp_f32 + invc[:, None]).astype(f32)
        conf_np = np.ascontiguousarray(conf_mask[rows].reshape(1, R), dtype=f32)
        in_maps.append(
            {
                "fTg": fTg_np,
                "fAn": fAn_np,
                "TAg": TAg_np,
                "fTc": fTc_np,
                "tTp": tTp_np,
                "W2": W2_np,
                "conf": conf_np,
                "rcc": rcc_np,
            }
        )
    return in_maps


def _run(centers1, features, targets, conf_mask, trace=False, trace_cores=None):
    in_maps = _prep_inputs(centers1, features, targets, conf_mask)
    nc = _get_nc()
    kwargs = {}
    if trace:
        # NTFF profiling under axon: shim the (absent) antenv.axon_hooks
        # module and skip the artifact bucket upload.
        import types
        import concourse.bass_utils as bass_utils

        if "antenv.axon_hooks" not in sys.modules:
            mod = types.ModuleType("antenv.axon_hooks")
            mod._hook = None

            def set_axon_ntff_profile_hook(h):
                mod._hook = h

            def get_axon_ntff_profile_hook():
                return mod._hook

            mod.set_axon_ntff_profile_hook = set_axon_ntff_profile_hook
            mod.get_axon_ntff_profile_hook = get_axon_ntff_profile_hook
            sys.modules["antenv.axon_hooks"] = mod
            from trn_agent_boot.trn_boot import _ntff_profile_via_ctypes

            set_axon_ntff_profile_hook(
                _ntff_profile_via_ctypes("/opt/axon/libaxon_pjrt.so")
            )
        bass_utils.upload_artifacts = lambda tmpdir: "local://" + tmpdir
        kwargs = {"trace": True}
        if trace_cores is not None:
            kwargs["trace_cores"] = trace_cores
    res = run_bass_kernel_spmd(nc, in_maps, core_ids=list(range(CORES)), **kwargs)
    num = 0.0
    den = 0.0
    for r in res.results:
        num += float(r["out"][0, 0])
        den += float(r["out"][0, 1])
    loss = np.array(num / den, dtype=np.float32)
    return loss, res


def kernel(centers1, features, targets, cls_num_list, conf_mask):
    loss, _ = _run(centers1, features, targets, conf_mask)
    return loss
